# revision 20
# baseline (speedup 1.0000x reference)
"""ConvNearestNeightbor Trainium2 kernel (hybrid ACT + custom fused DVE ops).

out[b, n*C+c, i, j] = max_k |x[b,c,i-r_k,j-c_k] - neighbors[n,c,k]|
over the 9 zero-padded 3x3 shifts (r_k, c_k).

Sharding: 8 cores = 4 batch-groups x 2 num-groups.
Per core: B_loc=4 batches, N_loc=16 codebook entries.
Partition layout: (nn in 0..3, c in 0..31) -> 128 partitions; codebook
tile nt in 0..3 selects n = nt*4+nn.

x layout: NO column padding -- a [128, 135*32] f32 tile, batch b interior
contiguous at rows 2+33b .. 33+33b (32x32), zero pad rows between
batches (vertical zero-padding is exact).  Every 3x3-shift "window" is
then a 1D contiguous crop at offset (2+33b-r)*32 - c, so input DMA is
4KB-run cheap and custom-DVE ops take windows as both operands.
Horizontal shifts wrap one column per row into the neighbouring row;
that single known column per shifted plane is overwritten with the
correct zero-pad value |n_k| (tiny strided tensor_scalar) before
folding.

Engine split per nt (measured rates: ACT 1.2G elem/s; DVE custom 1 elem/
cyc fused produce+abs+fold, TT bf16 2 elem/cyc):
 - ACT: planes {0,2,3,5,6,8} as |x-n| (Abs+bias, f32 in, bf16 out)
 - DVE: PAIR(k1,k7) per batch + ACC(k4) at 4096, column fixups, and
   6 bf16 tensor_tensor max folds at 4096.
Output: bf16 accumulator -> SWDGE cast DMA -> f32.
"""

import numpy as np

B, C, H, W = 16, 32, 32, 32
NUM = 32
NCORES = 8
BG, NG = 4, 2          # batch groups x num groups
B_LOC = B // BG        # 4
N_LOC = NUM // NG      # 16
NT = N_LOC // 4        # 4 codebook tiles of 4 n each
ROWS = 136             # guard + 4x(pad+32) + bottom pads/guards
FREE = ROWS * 32

K_ACT = (0, 2, 3, 5, 6, 8)

_module_cache = {}


def _register_ops():
    """Register the two fused abs-diff-max DVE ops (idempotent)."""
    import concourse.dve_ops as dve_ops
    from concourse.dve_spec import Spec, Src0, Src1, C0, C1, maxx, lower
    from concourse.dve_uop import DveOpSpec
    from concourse.dve_table_gen import dve_ver_for

    names = ("ABSD_MAX_PAIR_AK", "ABSD_MAX_ACC_AK")
    if names[0] in dve_ops._SUB_OPCODE_FOR_NAME:
        by_name = {op.name: op for op in dve_ops.OPS}
        return by_name[names[0]], by_name[names[1]]

    ver = dve_ver_for("TRN2")

    def mk(name, body, ref):
        spec = Spec(body=body, reference=ref)
        row = max(dve_ops._SUB_OPCODE_FOR_NAME.values()) + 1
        assert row < 0x20
        dve_ops._SUB_OPCODE_FOR_NAME[name] = row
        uops = lower(spec, ver=ver)
        sha = DveOpSpec(name=name, opcode=row, uops=uops, rd1_en=True).sha(ver)
        op = dve_ops.DveOp(name, spec, subdim=False, uops_sha={ver: sha})
        dve_ops.OPS.append(op)
        dve_ops.CUSTOM_DVE_SPECS[name] = spec
        return op

    pair = mk(
        names[0],
        maxx(maxx(Src0 - C0, C0 - Src0), maxx(Src1 - C1, C1 - Src1)),
        lambda in0, in1, s0, s1, imm2: np.maximum(
            np.abs(in0.astype(np.float32) - s0),
            np.abs(in1.astype(np.float32).reshape(in0.shape) - s1),
        ),
    )
    acc = mk(
        names[1],
        maxx(maxx(Src0 - C0, C0 - Src0), Src1),
        lambda in0, in1, s0, s1, imm2: np.maximum(
            np.abs(in0.astype(np.float32) - s0),
            in1.astype(np.float32).reshape(in0.shape),
        ),
    )
    return pair, acc


def _build_module():
    import concourse.bacc as bacc
    import concourse.mybir as mybir
    import concourse.tile as tile

    PAIR, ACC = _register_ops()

    dt = mybir.dt
    Alu = mybir.AluOpType
    AF = mybir.ActivationFunctionType

    nc = bacc.Bacc("TRN2", debug=False)
    x = nc.dram_tensor("x", [B_LOC, C, H, W], dt.float32, kind="ExternalInput")
    nb = nc.dram_tensor("neighbors", [N_LOC, C, 9], dt.float32, kind="ExternalInput")
    out = nc.dram_tensor(
        "out", [B_LOC, N_LOC * C, H, W], dt.float32, kind="ExternalOutput"
    )

    # shift k = (r+1)*3 + (c+1) with r, c in {-1, 0, 1}
    RC = [(r, c) for r in (-1, 0, 1) for c in (-1, 0, 1)]

    def wstart(b, k):
        r, c = RC[k]
        return (2 + 33 * b - r) * 32 - c

    # bad (wrapped) column of a c-shifted plane: c=+1 -> col 0, c=-1 -> col 31
    def badcol(k):
        c = RC[k][1]
        return None if c == 0 else (0 if c == 1 else 31)

    with tile.TileContext(nc) as tc:
        with (
            tc.tile_pool(name="const", bufs=1) as cpool,
            tc.tile_pool(name="pp", bufs=2) as ppool,
            tc.tile_pool(name="dp", bufs=2) as dpool,
            tc.tile_pool(name="fp", bufs=2) as fpool,
        ):
            # neighbors scalars first (tiny DMA; gates first ops)
            nbt = cpool.tile([128, NT * 9], dt.float32, tag="nbt")
            nb_src = nb.ap().rearrange("(t nn) c k -> (nn c) t k", nn=4)
            nc.sync.dma_start(nbt[:].rearrange("p (t k) -> p t k", t=NT), nb_src)
            nbneg = cpool.tile([128, NT * 9], dt.float32, tag="nbneg")
            nc.scalar.mul(nbneg[:], nbt[:], -1.0)
            nbabs = cpool.tile([128, NT * 9], dt.float32, tag="nbabs")
            nc.scalar.activation(nbabs[:], nbt[:], AF.Abs, scale=1.0)

            def nbcol(nt, k):
                return nbt[:, nt * 9 + k : nt * 9 + k + 1]

            def nbnegcol(nt, k):
                return nbneg[:, nt * 9 + k : nt * 9 + k + 1]

            def nbabscol(nt, k):
                return nbabs[:, nt * 9 + k : nt * 9 + k + 1]

            # padded x tile (bf16, SWDGE cast loads) and pad-row memsets
            xpad = cpool.tile([128, FREE], dt.bfloat16, tag="xpad")
            xr = xpad[:].rearrange("p (r w) -> p r w", r=ROWS)
            nc.vector.memset(xr[:, 0:2, :], 0.0)        # guard + top pad
            for b in range(1, B_LOC):
                nc.vector.memset(xr[:, 33 * b + 1 : 33 * b + 2, :], 0.0)
            nc.vector.memset(xr[:, ROWS - 3 : ROWS, :], 0.0)

            # interior loads (contiguous 1024 per (b, c)), SWDGE cast
            for b in range(B_LOC):
                for nn in range(4):
                    nc.gpsimd.dma_start(
                        xr[nn * 32 : (nn + 1) * 32, 2 + 33 * b : 34 + 33 * b, :],
                        x.ap()[b],
                    )

            def win(b, k):
                s = wstart(b, k)
                return xpad[:, s : s + 1024]

            def win4(k):
                # all-batch window [p, 4, 1024] (stride 33*32)
                s = wstart(0, k)
                return xpad[:, s : s + 4 * 1056].rearrange(
                    "p (b u) -> p b u", u=1056
                )[:, :, :1024]

            accs = {}
            out_v = out.ap().rearrange("b (t p) h w -> t p b (h w)", t=NT)

            # max(|n_k0|, |n_k6|) per nt: fixup value for the PAIR(k0,k6) slab
            nbabs_v = nbabs[:].rearrange("p (t k) -> p t k", t=NT)
            nbm06 = cpool.tile([128, NT], dt.float32, tag="nbm06")
            nc.vector.tensor_tensor(
                nbm06[:], nbabs_v[:, :, 0], nbabs_v[:, :, 6], Alu.max
            )
            nc.vector.tensor_tensor(
                nbm06[:], nbm06[:], nbabs_v[:, :, 3], Alu.max
            )
            nbm258 = cpool.tile([128, NT], dt.float32, tag="nbm258")
            nc.vector.tensor_tensor(
                nbm258[:], nbabs_v[:, :, 2], nbabs_v[:, :, 8], Alu.max
            )
            nc.vector.tensor_tensor(
                nbm258[:], nbm258[:], nbabs_v[:, :, 5], Alu.max
            )

            # nts 0,1 "heavy" (DVE takes k0,k6 too; ACT 4 planes),
            # nts 2,3 "light" (ACT 6 planes) -- keeps ACT ahead of the
            # DVE fold pipeline.
            HEAVY = (True, True, True, True)
            K_ACT_H = (2, 8, 5, 3, 4)
            K_ACT_L = (0, 2, 3, 5, 6, 8)

            ds_all = {}
            pslab2_all = {}

            def emit_act(nt):
                ks = K_ACT_H if HEAVY[nt] else K_ACT_L
                ds = []
                for ki, k in enumerate(ks):
                    d = dpool.tile(
                        [128, B_LOC * H * W], dt.bfloat16, tag=f"d{ki}",
                        name=f"d{ki}_{nt}",
                    )
                    nc.scalar.activation(
                        d[:].rearrange("p (b u) -> p b u", b=B_LOC),
                        win4(k), AF.Abs, bias=nbnegcol(nt, k), scale=1.0,
                    )
                    ds.append(d)
                ds_all[nt] = ds

            def emit_chains(nt):
                pslab = ppool.tile(
                    [128, B_LOC * H * W], dt.bfloat16, tag="pslab"
                )
                ps_v = pslab[:].rearrange("p (b u) -> p b u", b=B_LOC)
                p2 = ppool.tile(
                    [128, B_LOC * H * W], dt.bfloat16, tag="pslab2"
                )
                p2_v = p2[:].rearrange("p (b u) -> p b u", b=B_LOC)
                for b in range(B_LOC):
                    nc.vector._custom_dve(
                        PAIR, out=ps_v[:, b], in0=win(b, 1), in1=win(b, 7),
                        s0=nbcol(nt, 1), s1=nbcol(nt, 7),
                    )
                    nc.vector._custom_dve(
                        PAIR, out=p2_v[:, b], in0=win(b, 0), in1=win(b, 6),
                        s0=nbcol(nt, 0), s1=nbcol(nt, 6),
                    )
                pslab2_all[nt] = p2
                accs[nt] = pslab

            def fixup(view_owner, col, scol):
                dv = view_owner[:].rearrange(
                    "p (b h w) -> p b h w", b=B_LOC, h=H
                )[:, :, :, col : col + 1]
                nc.vector.tensor_scalar(dv, dv, 0.0, scol, Alu.mult, Alu.add)

            def emit_folds(nt):
                ks = K_ACT_H if HEAVY[nt] else K_ACT_L
                ds = ds_all.pop(nt)
                sz = [128, B_LOC * H * W]
                p2 = pslab2_all.pop(nt)
                # ds = [d2, d8, d5, d3, d4]; 2/8/5 wrap col 0;
                # p2 (k0,k6) and d3 wrap col 31
                e0 = fpool.tile(sz, dt.bfloat16, tag="s0", name=f"e0_{nt}")
                nc.vector.tensor_tensor(e0[:], ds[0][:], ds[1][:], Alu.max)
                e0b = fpool.tile(sz, dt.bfloat16, tag="s1", name=f"e0b_{nt}")
                nc.vector.tensor_tensor(e0b[:], e0[:], ds[2][:], Alu.max)
                fixup(e0b, 0, nbm258[:, nt : nt + 1])
                g = fpool.tile(sz, dt.bfloat16, tag="s2", name=f"g_{nt}")
                nc.vector.tensor_tensor(g[:], p2[:], ds[3][:], Alu.max)
                fixup(g, 31, nbm06[:, nt : nt + 1])
                a2 = fpool.tile(sz, dt.bfloat16, tag="s0", name=f"a2_{nt}")
                nc.vector.tensor_tensor(a2[:], ds[4][:], accs[nt][:], Alu.max)
                f = fpool.tile(sz, dt.bfloat16, tag="s1", name=f"f_{nt}")
                nc.vector.tensor_tensor(f[:], e0b[:], g[:], Alu.max)
                a3 = f
                aF = fpool.tile(sz, dt.bfloat16, tag="s2", name=f"aF_{nt}")
                a3v = a2[:].rearrange("p (b s) -> p b s", b=B_LOC)
                fv = f[:].rearrange("p (b s) -> p b s", b=B_LOC)
                del a3
                aFv = aF[:].rearrange("p (b s) -> p b s", b=B_LOC)
                for b in range(B_LOC):
                    nc.vector.tensor_tensor(
                        aFv[:, b], a3v[:, b], fv[:, b], Alu.max
                    )
                    nc.gpsimd.dma_start(out_v[nt][:, b], aFv[:, b])

            for nt in range(NT):
                emit_act(nt)
                emit_chains(nt)
                if nt >= 1:
                    emit_folds(nt - 1)
            emit_folds(NT - 1)

    nc.compile()
    return nc


def _get_module():
    if "nc" not in _module_cache:
        _module_cache["nc"] = _build_module()
    return _module_cache["nc"]


def _run(x, neighbors, trace=False):
    from concourse import bass_utils

    x = np.ascontiguousarray(x, dtype=np.float32)
    neighbors = np.ascontiguousarray(neighbors, dtype=np.float32)
    in_maps = []
    for core in range(NCORES):
        bg, ng = divmod(core, NG)
        in_maps.append(
            {
                "x": x[bg * B_LOC : (bg + 1) * B_LOC],
                "neighbors": neighbors[ng * N_LOC : (ng + 1) * N_LOC],
            }
        )
    res = bass_utils.run_bass_kernel_spmd(
        _get_module(), in_maps, core_ids=list(range(NCORES)), trace=trace
    )
    out = np.empty((B, NUM * C, H, W), dtype=np.float32)
    for core in range(NCORES):
        bg, ng = divmod(core, NG)
        out[bg * B_LOC : (bg + 1) * B_LOC, ng * N_LOC * C : (ng + 1) * N_LOC * C] = (
            res.results[core]["out"]
        )
    return out, res


def kernel(x, neighbors):
    out, _ = _run(x, neighbors, trace=False)
    return out
